# revision 3
# baseline (speedup 1.0000x reference)
"""Multi-head self-attention Trainium2 Bass kernel (8 NeuronCores).

Problem: B=4, S=2048, D=1024, H=16 heads x DH=64.
Sharding: data-parallel over batch (4) x tensor-parallel over head-groups (2)
-> 8 cores, each computing out[b, :, hg*512:(hg+1)*512].

Per-core algorithm (all matmuls float32r, 1 cycle/row on the PE):
  - Host supplies x[b]^T  [D, S]; projections contract D on partitions.
  - Q^T, K^T computed per head-pair [128, S] (two heads' 64 d-cols stacked,
    enabling row-tiled concurrent score matmuls at K=64).
  - Scores computed TRANSPOSED: S^T[t, qi] = (K^T tile).T @ Q^T, 128 keys x
    512 queries per matmul -> softmax never needs a P-transpose.
  - exp on ACT directly from PSUM (scale=1/8 fused); no max-subtract needed
    (scores ~ N(0,1); exp cannot overflow fp32).
  - Mask folded into V: V2 = mask * [V + bv | 1]; the 65th lhsT column makes
    the PV matmul emit the masked softmax denominator for free.
  - PV: out^T[d(+den), qi] accumulated over key tiles in PSUM.
  - Epilogue: PE-transpose out^T blocks back to [qi, d], multiply by
    1/denominator (per-partition scalar), DMA to HBM.
"""

import os
import sys
import types

for _p in ("/opt/trn_rl_repo", os.path.expanduser("~/.axon_site/_ro/trn_rl_repo")):
    if os.path.isdir(_p) and _p not in sys.path:
        sys.path.insert(0, _p)

import numpy as np

import concourse.bacc as bacc
import concourse.tile as tile
from concourse import mybir
from concourse.bass_utils import run_bass_kernel_spmd
from concourse.masks import make_identity

B, S, D = 4, 2048, 1024
H, DH = 16, 64
NCORES = 8
HEADS_PER_CORE = 8
PAIRS = 4          # head pairs per core
NT = S // 128      # 16 key tiles
NQC = S // 512     # 4 query chunks of 512
F32 = mybir.dt.float32
F32R = mybir.dt.float32r

_CACHE = {}


def _build_program():
    """Build the (single) SPMD Bass program run on every core."""
    nc = bacc.Bacc("TRN2", target_bir_lowering=False, debug=False,
                   num_devices=NCORES)

    xT = nc.dram_tensor("xT", [D, S], F32R, kind="ExternalInput")
    wq = nc.dram_tensor("wq", [D, 512], F32R, kind="ExternalInput")
    wk = nc.dram_tensor("wk", [D, 512], F32R, kind="ExternalInput")
    wv = nc.dram_tensor("wv", [D, 512], F32R, kind="ExternalInput")
    mcols = nc.dram_tensor("mcols", [128, NT], F32, kind="ExternalInput")
    bqc = nc.dram_tensor("bqc", [128, PAIRS], F32, kind="ExternalInput")
    bkc = nc.dram_tensor("bkc", [128, PAIRS], F32, kind="ExternalInput")
    bvrep = nc.dram_tensor("bvrep", [128, 512], F32, kind="ExternalInput")
    out = nc.dram_tensor("out", [S, 512], F32, kind="ExternalOutput")

    with tile.TileContext(nc) as tc:
        _emit(nc, tc, xT, wq, wk, wv, mcols, bqc, bkc, bvrep, out)
    nc.compile()
    return nc


def _emit(nc, tc, xT, wq, wk, wv, mcols, bqc, bkc, bvrep, out):
    from contextlib import ExitStack
    ctx = ExitStack()
    with ctx:
        consts = ctx.enter_context(tc.tile_pool(name="consts", bufs=1))
        xt_pool = ctx.enter_context(tc.tile_pool(name="xt", bufs=1))
        v2_pool = ctx.enter_context(tc.tile_pool(name="v2", bufs=1))
        qkt_pool = ctx.enter_context(tc.tile_pool(name="qkt", bufs=2))
        wchunk = ctx.enter_context(tc.tile_pool(name="wchunk", bufs=6))
        e_pool = ctx.enter_context(tc.tile_pool(name="e", bufs=3))
        ot_sb = ctx.enter_context(tc.tile_pool(name="otsb", bufs=3))
        den_pool = ctx.enter_context(tc.tile_pool(name="den", bufs=4))
        out_pool = ctx.enter_context(tc.tile_pool(name="outp", bufs=4))
        stage = ctx.enter_context(tc.tile_pool(name="stage", bufs=3))
        ps_s = ctx.enter_context(tc.tile_pool(name="ps_s", bufs=2, space="PSUM"))
        ps_ot = ctx.enter_context(tc.tile_pool(name="ps_ot", bufs=4, space="PSUM"))
        dscratch = ctx.enter_context(tc.tile_pool(name="dscr", bufs=4, space="DRAM"))

        # ---- constants / resident tensors ----
        ident = consts.tile([128, 128], F32)
        make_identity(nc, ident[:])
        m_sb = consts.tile([128, NT], F32)
        nc.sync.dma_start(out=m_sb[:], in_=mcols[:])
        bq_sb = consts.tile([128, PAIRS], F32)
        nc.sync.dma_start(out=bq_sb[:], in_=bqc[:])
        bk_sb = consts.tile([128, PAIRS], F32)
        nc.sync.dma_start(out=bk_sb[:], in_=bkc[:])
        bv_sb = consts.tile([128, 512], F32)
        nc.sync.dma_start(out=bv_sb[:], in_=bvrep[:])
        ones8 = consts.tile([128, HEADS_PER_CORE], F32)
        nc.vector.memset(ones8[:], 1.0)
        # warm the exp table early (one-time ~2.7us load)
        warm = consts.tile([128, 16], F32)
        nc.vector.memset(warm[:], 0.0)
        nc.scalar.activation(warm[:], warm[:],
                             mybir.ActivationFunctionType.Exp, scale=1.0)

        # x^T resident: [128, 8, 2048] (D chunk k on dim 1)
        xt = xt_pool.tile([128, D // 128, S], F32R)
        xTr = xT.rearrange("(k p) t -> k p t", p=128)
        for k in range(D // 128):
            nc.sync.dma_start(out=xt[:, k, :], in_=xTr[k])

        # Wv resident: [128, 8, 512]
        wv_sb = consts.tile([128, D // 128, 512], F32R)
        wvr = wv.rearrange("(k p) n -> k p n", p=128)
        for k in range(D // 128):
            nc.sync.dma_start(out=wv_sb[:, k, :], in_=wvr[k])

        # ---- V projection + V2 staging (all heads) ----
        # V2[t-tile i] = [128, 8*65]: per head [V*m + bv*m | m]
        v2 = v2_pool.tile([128, NT, HEADS_PER_CORE * 65], F32R)
        for i in range(NT):
            pv = ps_s.tile([128, 1024], F32, tag="s")
            for k in range(D // 128):
                nc.tensor.matmul(
                    pv[:, 0:512],
                    xt[:, k, i * 128:(i + 1) * 128],
                    wv_sb[:, k, :],
                    start=(k == 0), stop=(k == D // 128 - 1),
                )
            vb = stage.tile([128, 512], F32, tag="vstage")
            nc.vector.tensor_tensor(out=vb[:], in0=pv[:, 0:512], in1=bv_sb[:],
                                    op=mybir.AluOpType.add)
            v2i = v2[:, i, :].rearrange("p (h c) -> p h c", c=65)
            nc.vector.tensor_scalar_mul(
                v2i[:, :, 0:64],
                vb[:].rearrange("p (h c) -> p h c", c=64),
                m_sb[:, i:i + 1],
            )
            nc.vector.tensor_scalar_mul(v2i[:, :, 64], ones8[:],
                                        m_sb[:, i:i + 1])

        # ---- per head-pair pipeline ----
        wqr = wq.rearrange("(k p) n -> k p n", p=128)
        wkr = wk.rearrange("(k p) n -> k p n", p=128)
        for p in range(PAIRS):
            # -- Q^T / K^T projection for this pair: [128 dcols, 2048] --
            qt = qkt_pool.tile([128, S], F32R, tag="qt")
            kt = qkt_pool.tile([128, S], F32R, tag="kt")
            for tq in range(4):  # t quarters of 512
                pp = ps_s.tile([128, 1024], F32, tag="s")
                for k in range(D // 128):
                    wq_c = wchunk.tile([128, 128], F32R, tag="w")
                    nc.sync.dma_start(out=wq_c[:],
                                      in_=wqr[k, :, p * 128:(p + 1) * 128])
                    nc.tensor.matmul(
                        pp[:, 0:512], wq_c[:],
                        xt[:, k, tq * 512:(tq + 1) * 512],
                        start=(k == 0), stop=(k == D // 128 - 1),
                    )
                for k in range(D // 128):
                    wk_c = wchunk.tile([128, 128], F32R, tag="w")
                    nc.sync.dma_start(out=wk_c[:],
                                      in_=wkr[k, :, p * 128:(p + 1) * 128])
                    nc.tensor.matmul(
                        pp[:, 512:1024], wk_c[:],
                        xt[:, k, tq * 512:(tq + 1) * 512],
                        start=(k == 0), stop=(k == D // 128 - 1),
                    )
                nc.vector.tensor_scalar_add(
                    qt[:, tq * 512:(tq + 1) * 512], pp[:, 0:512],
                    bq_sb[:, p:p + 1])
                nc.vector.tensor_scalar_add(
                    kt[:, tq * 512:(tq + 1) * 512], pp[:, 512:1024],
                    bk_sb[:, p:p + 1])

            # -- attention core --
            otA = ot_sb.tile([65, S], F32, tag="ot_sb")
            otB = ot_sb.tile([65, S], F32, tag="ot_sb")
            for qc in range(NQC):
                oA = ps_ot.tile([65, 512], F32, tag="ot")
                oB = ps_ot.tile([65, 512], F32, tag="ot")
                for i in range(NT):
                    sp = ps_s.tile([128, 1024], F32, tag="s")
                    # scores^T for both heads (row groups 0 / 64, concurrent)
                    nc.tensor.matmul(
                        sp[:, 0:512],
                        kt[0:64, i * 128:(i + 1) * 128],
                        qt[0:64, qc * 512:(qc + 1) * 512],
                        start=True, stop=True,
                    )
                    nc.tensor.matmul(
                        sp[:, 512:1024],
                        kt[64:128, i * 128:(i + 1) * 128],
                        qt[64:128, qc * 512:(qc + 1) * 512],
                        start=True, stop=True,
                    )
                    ep = e_pool.tile([128, 1024], F32R, tag="e")
                    nc.scalar.activation(ep[:], sp[:],
                                         mybir.ActivationFunctionType.Exp,
                                         scale=0.125)
                    v2i = v2[:, i, :]
                    hA = 2 * p
                    hB = 2 * p + 1
                    nc.tensor.matmul(oA[:], v2i[:, hA * 65:(hA + 1) * 65],
                                     ep[:, 0:512],
                                     start=(i == 0), stop=(i == NT - 1))
                    nc.tensor.matmul(oB[:], v2i[:, hB * 65:(hB + 1) * 65],
                                     ep[:, 512:1024],
                                     start=(i == 0), stop=(i == NT - 1))
                nc.vector.tensor_copy(otA[:, qc * 512:(qc + 1) * 512], oA[:])
                nc.vector.tensor_copy(otB[:, qc * 512:(qc + 1) * 512], oB[:])

            # -- denominators -> [128, 16] transposed + reciprocal --
            rTs = []
            for hs, ot_t in ((0, otA), (1, otB)):
                dscr = dscratch.tile([S], F32, tag="dscr")
                nc.sync.dma_start(out=dscr[:], in_=ot_t[64:65, :])
                denT = den_pool.tile([128, NT], F32, tag="denT")
                nc.sync.dma_start(
                    out=denT[:],
                    in_=dscr.rearrange("(j q) -> q j", q=128),
                )
                rT = den_pool.tile([128, NT], F32, tag="rT")
                nc.vector.reciprocal(rT[:], denT[:])
                rTs.append(rT)

            # -- transpose back + normalize + store --
            for j in range(NT):
                ott = out_pool.tile([128, 128], F32, tag="outt")
                for hs, ot_t in ((0, otA), (1, otB)):
                    ptr = ps_ot.tile([128, 64], F32, tag="ot")
                    nc.tensor.transpose(
                        ptr[:], ot_t[0:64, j * 128:(j + 1) * 128],
                        ident[0:64, 0:64],
                    )
                    nc.vector.tensor_scalar_mul(
                        ott[:, hs * 64:(hs + 1) * 64], ptr[:],
                        rTs[hs][:, j:j + 1],
                    )
                nc.sync.dma_start(
                    out=out[j * 128:(j + 1) * 128, p * 128:(p + 1) * 128],
                    in_=ott[:],
                )


def _prep_core_inputs(c, x, mask, Wq, bq, Wk, bk, Wv, bv):
    b, hg = divmod(c, 2)
    cs = slice(hg * 512, (hg + 1) * 512)
    xTb = np.ascontiguousarray(x[b].T, dtype=np.float32)
    m = np.ascontiguousarray(mask[b].reshape(NT, 128).T, dtype=np.float32)
    bqc = np.ascontiguousarray(bq[cs].reshape(PAIRS, 128).T, dtype=np.float32)
    bkc = np.ascontiguousarray(bk[cs].reshape(PAIRS, 128).T, dtype=np.float32)
    bvrep = np.ascontiguousarray(
        np.broadcast_to(bv[cs][None, :], (128, 512)), dtype=np.float32)
    return {
        "xT": xTb,
        "wq": np.ascontiguousarray(Wq[:, cs], dtype=np.float32),
        "wk": np.ascontiguousarray(Wk[:, cs], dtype=np.float32),
        "wv": np.ascontiguousarray(Wv[:, cs], dtype=np.float32),
        "mcols": m,
        "bqc": bqc,
        "bkc": bkc,
        "bvrep": bvrep,
    }


def kernel(x, mask, Wq, bq, Wk, bk, Wv, bv, _trace=False, _trace_kwargs=None):
    x = np.asarray(x, dtype=np.float32)
    mask = np.asarray(mask, dtype=np.float32)
    assert x.shape == (B, S, D) and mask.shape == (B, S)
    # every batch row must keep at least one unmasked key (softmax denominator)
    assert (mask.reshape(B, S).sum(axis=1) > 0).all()

    if "nc" not in _CACHE:
        _CACHE["nc"] = _build_program()
    nc = _CACHE["nc"]

    in_maps = [_prep_core_inputs(c, x, mask, np.asarray(Wq, np.float32),
                                 np.asarray(bq, np.float32),
                                 np.asarray(Wk, np.float32),
                                 np.asarray(bk, np.float32),
                                 np.asarray(Wv, np.float32),
                                 np.asarray(bv, np.float32))
               for c in range(NCORES)]
    kwargs = {}
    if _trace:
        kwargs["trace"] = True
        kwargs.update(_trace_kwargs or {})
    res = run_bass_kernel_spmd(nc, in_maps, core_ids=list(range(NCORES)),
                               **kwargs)
    full = np.empty((B, S, H * DH), dtype=np.float32)
    for c in range(NCORES):
        b, hg = divmod(c, 2)
        full[b, :, hg * 512:(hg + 1) * 512] = res.results[c]["out"]
    if _trace:
        kernel.last_exec_time_ns = res.exec_time_ns
        kernel.last_results = res
    return full


# revision 4
# speedup vs baseline: 1.0844x; 1.0844x over previous
"""Multi-head self-attention Trainium2 Bass kernel (8 NeuronCores).

Problem: B=4, S=2048, D=1024, H=16 heads x DH=64.
Sharding: data-parallel over batch (4) x tensor-parallel over head-groups (2)
-> 8 cores, each computing out[b, :, hg*512:(hg+1)*512].

Per-core algorithm (matmul operands bf16 -> full PE stream rate; fp32 PSUM):
  - Host supplies x[b]^T  [D, S]; projections contract D on partitions.
  - Q^T, K^T computed per head-pair [128, S] (two heads' 64 d-cols stacked).
  - Scores computed TRANSPOSED: S^T[t, qi] = (K^T tile).T @ Q^T, 128 keys x
    512 queries per matmul -> softmax never needs a P-transpose.
  - exp on ACT directly from PSUM (scale=1/8 fused); no max-subtract needed
    (scores ~ N(0,1); exp cannot overflow fp32).
  - Mask folded into V: V2 = mask * [V + bv | 1]; the 65th lhsT column makes
    the PV matmul emit the masked softmax denominator for free.
  - PV: out^T[d(+den), qi] accumulated over key tiles in PSUM (fp32).
  - Epilogue: PE-transpose out^T blocks back to [qi, d], multiply by
    1/denominator (fp32, per-partition scalar), DMA to HBM.
"""

import os
import sys

for _p in ("/opt/trn_rl_repo", os.path.expanduser("~/.axon_site/_ro/trn_rl_repo")):
    if os.path.isdir(_p) and _p not in sys.path:
        sys.path.insert(0, _p)

import ml_dtypes
import numpy as np

import concourse.bacc as bacc
import concourse.tile as tile
from concourse import mybir
from concourse.bass_utils import run_bass_kernel_spmd
from concourse.masks import make_identity

B, S, D = 4, 2048, 1024
H, DH = 16, 64
NCORES = 8
HEADS_PER_CORE = 8
PAIRS = 4          # head pairs per core
NT = S // 128      # 16 key tiles
NQC = S // 512     # 4 query chunks of 512
F32 = mybir.dt.float32
CDT = mybir.dt.bfloat16          # matmul-operand compute dtype
CNP = ml_dtypes.bfloat16

_CACHE = {}


def _build_program():
    """Build the (single) SPMD Bass program run on every core."""
    nc = bacc.Bacc("TRN2", target_bir_lowering=False, debug=False,
                   num_devices=NCORES)

    xT = nc.dram_tensor("xT", [D, S], CDT, kind="ExternalInput")
    wq = nc.dram_tensor("wq", [D, 512], CDT, kind="ExternalInput")
    wk = nc.dram_tensor("wk", [D, 512], CDT, kind="ExternalInput")
    wv = nc.dram_tensor("wv", [D, 512], CDT, kind="ExternalInput")
    mcols = nc.dram_tensor("mcols", [128, NT], F32, kind="ExternalInput")
    bqc = nc.dram_tensor("bqc", [128, PAIRS], F32, kind="ExternalInput")
    bkc = nc.dram_tensor("bkc", [128, PAIRS], F32, kind="ExternalInput")
    bvrep = nc.dram_tensor("bvrep", [128, 512], F32, kind="ExternalInput")
    out = nc.dram_tensor("out", [S, 512], F32, kind="ExternalOutput")

    with tile.TileContext(nc) as tc:
        _emit(nc, tc, xT, wq, wk, wv, mcols, bqc, bkc, bvrep, out)
    nc.compile()
    return nc


def _emit(nc, tc, xT, wq, wk, wv, mcols, bqc, bkc, bvrep, out):
    from contextlib import ExitStack
    ctx = ExitStack()
    with ctx:
        consts = ctx.enter_context(tc.tile_pool(name="consts", bufs=1))
        xt_pool = ctx.enter_context(tc.tile_pool(name="xt", bufs=1))
        v2_pool = ctx.enter_context(tc.tile_pool(name="v2", bufs=1))
        qkt_pool = ctx.enter_context(tc.tile_pool(name="qkt", bufs=2))
        wchunk = ctx.enter_context(tc.tile_pool(name="wchunk", bufs=6))
        e_pool = ctx.enter_context(tc.tile_pool(name="e", bufs=3))
        ot_sb = ctx.enter_context(tc.tile_pool(name="otsb", bufs=3))
        den_pool = ctx.enter_context(tc.tile_pool(name="den", bufs=4))
        out_pool = ctx.enter_context(tc.tile_pool(name="outp", bufs=4))
        stage = ctx.enter_context(tc.tile_pool(name="stage", bufs=3))
        ps_s = ctx.enter_context(tc.tile_pool(name="ps_s", bufs=2, space="PSUM"))
        ps_ot = ctx.enter_context(tc.tile_pool(name="ps_ot", bufs=4, space="PSUM"))
        dscratch = ctx.enter_context(tc.tile_pool(name="dscr", bufs=4, space="DRAM"))

        # ---- constants / resident tensors ----
        ident = consts.tile([128, 128], CDT)
        make_identity(nc, ident[:])
        m_sb = consts.tile([128, NT], F32)
        nc.sync.dma_start(out=m_sb[:], in_=mcols[:])
        bq_sb = consts.tile([128, PAIRS], F32)
        nc.sync.dma_start(out=bq_sb[:], in_=bqc[:])
        bk_sb = consts.tile([128, PAIRS], F32)
        nc.sync.dma_start(out=bk_sb[:], in_=bkc[:])
        bv_sb = consts.tile([128, 512], F32)
        nc.sync.dma_start(out=bv_sb[:], in_=bvrep[:])
        ones8 = consts.tile([128, HEADS_PER_CORE], F32)
        nc.vector.memset(ones8[:], 1.0)
        # warm the exp table early (one-time ~2.7us load)
        warm = consts.tile([128, 16], F32)
        nc.vector.memset(warm[:], 0.0)
        nc.scalar.activation(warm[:], warm[:],
                             mybir.ActivationFunctionType.Exp, scale=1.0)

        # x^T resident: [128, 8, 2048] (D chunk k on dim 1)
        xt = xt_pool.tile([128, D // 128, S], CDT)
        xTr = xT.rearrange("(k p) t -> k p t", p=128)
        for k in range(D // 128):
            nc.sync.dma_start(out=xt[:, k, :], in_=xTr[k])

        # Wv resident: [128, 8, 512]
        wv_sb = consts.tile([128, D // 128, 512], CDT)
        wvr = wv.rearrange("(k p) n -> k p n", p=128)
        for k in range(D // 128):
            nc.sync.dma_start(out=wv_sb[:, k, :], in_=wvr[k])

        # ---- V projection + V2 staging (all heads) ----
        # V2[t-tile i] = [128, 8*65]: per head [V*m + bv*m | m]
        v2 = v2_pool.tile([128, NT, HEADS_PER_CORE * 65], CDT)
        for i in range(NT):
            pv = ps_s.tile([128, 1024], F32, tag="s")
            for k in range(D // 128):
                nc.tensor.matmul(
                    pv[:, 0:512],
                    xt[:, k, i * 128:(i + 1) * 128],
                    wv_sb[:, k, :],
                    start=(k == 0), stop=(k == D // 128 - 1),
                )
            vb = stage.tile([128, 512], F32, tag="vstage")
            nc.vector.tensor_tensor(out=vb[:], in0=pv[:, 0:512], in1=bv_sb[:],
                                    op=mybir.AluOpType.add)
            v2i = v2[:, i, :].rearrange("p (h c) -> p h c", c=65)
            nc.vector.tensor_scalar_mul(
                v2i[:, :, 0:64],
                vb[:].rearrange("p (h c) -> p h c", c=64),
                m_sb[:, i:i + 1],
            )
            nc.vector.tensor_scalar_mul(v2i[:, :, 64], ones8[:],
                                        m_sb[:, i:i + 1])

        # ---- per head-pair pipeline ----
        wqr = wq.rearrange("(k p) n -> k p n", p=128)
        wkr = wk.rearrange("(k p) n -> k p n", p=128)
        for p in range(PAIRS):
            # -- Q^T / K^T projection for this pair: [128 dcols, 2048] --
            qt = qkt_pool.tile([128, S], CDT, tag="qt")
            kt = qkt_pool.tile([128, S], CDT, tag="kt")
            for tq in range(4):  # t quarters of 512
                pp = ps_s.tile([128, 1024], F32, tag="s")
                for k in range(D // 128):
                    wq_c = wchunk.tile([128, 128], CDT, tag="w")
                    nc.sync.dma_start(out=wq_c[:],
                                      in_=wqr[k, :, p * 128:(p + 1) * 128])
                    nc.tensor.matmul(
                        pp[:, 0:512], wq_c[:],
                        xt[:, k, tq * 512:(tq + 1) * 512],
                        start=(k == 0), stop=(k == D // 128 - 1),
                    )
                for k in range(D // 128):
                    wk_c = wchunk.tile([128, 128], CDT, tag="w")
                    nc.sync.dma_start(out=wk_c[:],
                                      in_=wkr[k, :, p * 128:(p + 1) * 128])
                    nc.tensor.matmul(
                        pp[:, 512:1024], wk_c[:],
                        xt[:, k, tq * 512:(tq + 1) * 512],
                        start=(k == 0), stop=(k == D // 128 - 1),
                    )
                nc.vector.tensor_scalar_add(
                    qt[:, tq * 512:(tq + 1) * 512], pp[:, 0:512],
                    bq_sb[:, p:p + 1])
                nc.vector.tensor_scalar_add(
                    kt[:, tq * 512:(tq + 1) * 512], pp[:, 512:1024],
                    bk_sb[:, p:p + 1])

            # -- attention core --
            otA = ot_sb.tile([65, S], CDT, tag="ot_sb")
            otB = ot_sb.tile([65, S], CDT, tag="ot_sb")
            denA = den_pool.tile([65, S], F32, tag="den64")
            denB = den_pool.tile([65, S], F32, tag="den64")
            for qc in range(NQC):
                oA = ps_ot.tile([65, 512], F32, tag="ot")
                oB = ps_ot.tile([65, 512], F32, tag="ot")
                for i in range(NT):
                    sp = ps_s.tile([128, 1024], F32, tag="s")
                    # scores^T for both heads (row groups 0 / 64, concurrent)
                    nc.tensor.matmul(
                        sp[:, 0:512],
                        kt[0:64, i * 128:(i + 1) * 128],
                        qt[0:64, qc * 512:(qc + 1) * 512],
                        start=True, stop=True,
                    )
                    nc.tensor.matmul(
                        sp[:, 512:1024],
                        kt[64:128, i * 128:(i + 1) * 128],
                        qt[64:128, qc * 512:(qc + 1) * 512],
                        start=True, stop=True,
                    )
                    ep = e_pool.tile([128, 1024], CDT, tag="e")
                    nc.scalar.activation(ep[:], sp[:],
                                         mybir.ActivationFunctionType.Exp,
                                         scale=0.125)
                    v2i = v2[:, i, :]
                    hA = 2 * p
                    hB = 2 * p + 1
                    nc.tensor.matmul(oA[:], v2i[:, hA * 65:(hA + 1) * 65],
                                     ep[:, 0:512],
                                     start=(i == 0), stop=(i == NT - 1))
                    nc.tensor.matmul(oB[:], v2i[:, hB * 65:(hB + 1) * 65],
                                     ep[:, 512:1024],
                                     start=(i == 0), stop=(i == NT - 1))
                qs = slice(qc * 512, (qc + 1) * 512)
                nc.vector.tensor_copy(otA[0:64, qs], oA[0:64, :])
                nc.vector.tensor_copy(otB[0:64, qs], oB[0:64, :])
                nc.vector.tensor_copy(denA[64:65, qs], oA[64:65, :])
                nc.vector.tensor_copy(denB[64:65, qs], oB[64:65, :])

            # -- denominators -> [128, 16] transposed + reciprocal (fp32) --
            rTs = []
            for hs, den_t in ((0, denA), (1, denB)):
                dscr = dscratch.tile([S], F32, tag="dscr")
                nc.sync.dma_start(out=dscr[:], in_=den_t[64:65, :])
                denT = den_pool.tile([128, NT], F32, tag="denT")
                nc.sync.dma_start(
                    out=denT[:],
                    in_=dscr.rearrange("(j q) -> q j", q=128),
                )
                rT = den_pool.tile([128, NT], F32, tag="rT")
                nc.vector.reciprocal(rT[:], denT[:])
                rTs.append(rT)

            # -- transpose back + normalize + store --
            for j in range(NT):
                ott = out_pool.tile([128, 128], F32, tag="outt")
                for hs, ot_t in ((0, otA), (1, otB)):
                    ptr = ps_ot.tile([128, 64], CDT, tag="ot")
                    nc.tensor.transpose(
                        ptr[:], ot_t[0:64, j * 128:(j + 1) * 128],
                        ident[0:64, 0:64],
                    )
                    nc.vector.tensor_scalar_mul(
                        ott[:, hs * 64:(hs + 1) * 64], ptr[:],
                        rTs[hs][:, j:j + 1],
                    )
                nc.sync.dma_start(
                    out=out[j * 128:(j + 1) * 128, p * 128:(p + 1) * 128],
                    in_=ott[:],
                )


def _prep_core_inputs(c, x, mask, Wq, bq, Wk, bk, Wv, bv):
    b, hg = divmod(c, 2)
    cs = slice(hg * 512, (hg + 1) * 512)
    xTb = np.ascontiguousarray(x[b].T).astype(CNP)
    m = np.ascontiguousarray(mask[b].reshape(NT, 128).T, dtype=np.float32)
    bqc = np.ascontiguousarray(bq[cs].reshape(PAIRS, 128).T, dtype=np.float32)
    bkc = np.ascontiguousarray(bk[cs].reshape(PAIRS, 128).T, dtype=np.float32)
    bvrep = np.ascontiguousarray(
        np.broadcast_to(bv[cs][None, :], (128, 512)), dtype=np.float32)
    return {
        "xT": xTb,
        "wq": np.ascontiguousarray(Wq[:, cs]).astype(CNP),
        "wk": np.ascontiguousarray(Wk[:, cs]).astype(CNP),
        "wv": np.ascontiguousarray(Wv[:, cs]).astype(CNP),
        "mcols": m,
        "bqc": bqc,
        "bkc": bkc,
        "bvrep": bvrep,
    }


def kernel(x, mask, Wq, bq, Wk, bk, Wv, bv, _trace=False, _trace_kwargs=None):
    x = np.asarray(x, dtype=np.float32)
    mask = np.asarray(mask, dtype=np.float32)
    assert x.shape == (B, S, D) and mask.shape == (B, S)
    # every batch row must keep at least one unmasked key (softmax denominator)
    assert (mask.reshape(B, S).sum(axis=1) > 0).all()

    if "nc" not in _CACHE:
        _CACHE["nc"] = _build_program()
    nc = _CACHE["nc"]

    in_maps = [_prep_core_inputs(c, x, mask, np.asarray(Wq, np.float32),
                                 np.asarray(bq, np.float32),
                                 np.asarray(Wk, np.float32),
                                 np.asarray(bk, np.float32),
                                 np.asarray(Wv, np.float32),
                                 np.asarray(bv, np.float32))
               for c in range(NCORES)]
    kwargs = {}
    if _trace:
        kwargs["trace"] = True
        kwargs.update(_trace_kwargs or {})
    res = run_bass_kernel_spmd(nc, in_maps, core_ids=list(range(NCORES)),
                               **kwargs)
    full = np.empty((B, S, H * DH), dtype=np.float32)
    for c in range(NCORES):
        b, hg = divmod(c, 2)
        full[b, :, hg * 512:(hg + 1) * 512] = res.results[c]["out"]
    if _trace:
        kernel.last_exec_time_ns = res.exec_time_ns
        kernel.last_results = res
    return full


# revision 7
# speedup vs baseline: 1.2513x; 1.1539x over previous
"""Multi-head self-attention Trainium2 Bass kernel (8 NeuronCores).

Problem: B=4, S=2048, D=1024, H=16 heads x DH=64.
Sharding: data-parallel over batch (4) x tensor-parallel over head-groups (2)
-> 8 cores, each computing out[b, :, hg*512:(hg+1)*512].

Per-core algorithm (matmul operands bf16 -> full PE stream rate; fp32 PSUM):
  - Host supplies x[b]^T  [D, S]; projections contract D on partitions.
  - Q^T, K^T computed per head-pair [128, S] (two heads' 64 d-cols stacked).
  - Scores computed TRANSPOSED: S^T[t, qi] = (K^T tile).T @ Q^T, 128 keys x
    512 queries per matmul -> softmax never needs a P-transpose.
  - exp on ACT directly from PSUM (scale=1/8 fused); no max-subtract needed
    (scores ~ N(0,1); exp cannot overflow fp32).
  - Mask folded into V: V2 = mask * [V + bv | 1]; the 65th lhsT column makes
    the PV matmul emit the masked softmax denominator for free.
  - PV: out^T[d(+den), qi] accumulated over key tiles in PSUM (fp32).
  - Epilogue: PE-transpose out^T blocks back to [qi, d], multiply by
    1/denominator (fp32, per-partition scalar), DMA to HBM.
"""

import os
import sys

for _p in ("/opt/trn_rl_repo", os.path.expanduser("~/.axon_site/_ro/trn_rl_repo")):
    if os.path.isdir(_p) and _p not in sys.path:
        sys.path.insert(0, _p)

import ml_dtypes
import numpy as np

import concourse.bacc as bacc
import concourse.tile as tile
from concourse import mybir
from concourse.bass_utils import run_bass_kernel_spmd
from concourse.masks import make_identity

B, S, D = 4, 2048, 1024
H, DH = 16, 64
NCORES = 8
HEADS_PER_CORE = 8
PAIRS = 4          # head pairs per core
NT = S // 128      # 16 key tiles
NQC = S // 512     # 4 query chunks of 512
F32 = mybir.dt.float32
CDT = mybir.dt.bfloat16          # matmul-operand compute dtype
CNP = ml_dtypes.bfloat16

_CACHE = {}


def _build_program():
    """Build the (single) SPMD Bass program run on every core."""
    nc = bacc.Bacc("TRN2", target_bir_lowering=False, debug=False,
                   num_devices=NCORES)

    xT = nc.dram_tensor("xT", [D, S], CDT, kind="ExternalInput")
    wq = nc.dram_tensor("wq", [D, 512], CDT, kind="ExternalInput")
    wk = nc.dram_tensor("wk", [D, 512], CDT, kind="ExternalInput")
    wv = nc.dram_tensor("wv", [D, 512], CDT, kind="ExternalInput")
    mcols = nc.dram_tensor("mcols", [128, NT], F32, kind="ExternalInput")
    bqc = nc.dram_tensor("bqc", [128, PAIRS], F32, kind="ExternalInput")
    bkc = nc.dram_tensor("bkc", [128, PAIRS], F32, kind="ExternalInput")
    bvrep = nc.dram_tensor("bvrep", [128, 512], F32, kind="ExternalInput")
    out = nc.dram_tensor("out", [S, 512], F32, kind="ExternalOutput")

    with tile.TileContext(nc) as tc:
        _emit(nc, tc, xT, wq, wk, wv, mcols, bqc, bkc, bvrep, out)
    nc.compile()
    return nc


def _emit(nc, tc, xT, wq, wk, wv, mcols, bqc, bkc, bvrep, out):
    from contextlib import ExitStack
    ctx = ExitStack()
    with ctx:
        consts = ctx.enter_context(tc.tile_pool(name="consts", bufs=1))
        xt_pool = ctx.enter_context(tc.tile_pool(name="xt", bufs=1))
        v2_pool = ctx.enter_context(tc.tile_pool(name="v2", bufs=1))
        qkt_pool = ctx.enter_context(tc.tile_pool(name="qkt", bufs=2))
        wchunk = ctx.enter_context(tc.tile_pool(name="wchunk", bufs=6))
        e_pool = ctx.enter_context(tc.tile_pool(name="e", bufs=3))
        ot_sb = ctx.enter_context(tc.tile_pool(name="otsb", bufs=3))
        den_pool = ctx.enter_context(tc.tile_pool(name="den", bufs=4))
        out_pool = ctx.enter_context(tc.tile_pool(name="outp", bufs=4))
        stage = ctx.enter_context(tc.tile_pool(name="stage", bufs=3))
        # PSUM budget (8 banks): ps_s 2x[128,1024]=4, ps_ot 2x[65,512]=2,
        # ps_proj 1x[128,1024]=2. Projections get their own banks so they
        # overlap the (ACT-bound) attention phase instead of serializing.
        ps_s = ctx.enter_context(tc.tile_pool(name="ps_s", bufs=2, space="PSUM"))
        ps_ot = ctx.enter_context(tc.tile_pool(name="ps_ot", bufs=2, space="PSUM"))
        ps_proj = ctx.enter_context(tc.tile_pool(name="ps_proj", bufs=1, space="PSUM"))
        dscratch = ctx.enter_context(tc.tile_pool(name="dscr", bufs=4, space="DRAM"))

        # ---- constants / resident tensors ----
        ident = consts.tile([128, 128], CDT)
        make_identity(nc, ident[:])
        m_sb = consts.tile([128, NT], F32)
        nc.sync.dma_start(out=m_sb[:], in_=mcols[:])
        bq_sb = consts.tile([128, PAIRS], F32)
        nc.sync.dma_start(out=bq_sb[:], in_=bqc[:])
        bk_sb = consts.tile([128, PAIRS], F32)
        nc.sync.dma_start(out=bk_sb[:], in_=bkc[:])
        bv_sb = consts.tile([128, 512], F32)
        nc.sync.dma_start(out=bv_sb[:], in_=bvrep[:])
        ones8 = consts.tile([128, HEADS_PER_CORE], F32)
        nc.vector.memset(ones8[:], 1.0)
        # warm the exp table early (one-time ~2.7us load)
        warm = consts.tile([128, 16], F32)
        nc.vector.memset(warm[:], 0.0)
        nc.scalar.activation(warm[:], warm[:],
                             mybir.ActivationFunctionType.Exp, scale=1.0)

        # x^T resident: [128, 8, 2048] (D chunk k on dim 1)
        xt = xt_pool.tile([128, D // 128, S], CDT)
        xTr = xT.rearrange("(k p) t -> k p t", p=128)
        for k in range(D // 128):
            nc.sync.dma_start(out=xt[:, k, :], in_=xTr[k])

        # Wv resident: [128, 8, 512]
        wv_sb = consts.tile([128, D // 128, 512], CDT)
        wvr = wv.rearrange("(k p) n -> k p n", p=128)
        for k in range(D // 128):
            nc.sync.dma_start(out=wv_sb[:, k, :], in_=wvr[k])

        # ---- V projection + V2 staging (all heads) ----
        # V2[t-tile i] = [128, 8*65]: per head [V*m + bv*m | m]
        v2 = v2_pool.tile([128, NT, HEADS_PER_CORE * 65], CDT)
        for i in range(NT):
            pv = ps_proj.tile([128, 1024], F32, tag="proj")
            for k in range(D // 128):
                nc.tensor.matmul(
                    pv[:, 0:512],
                    xt[:, k, i * 128:(i + 1) * 128],
                    wv_sb[:, k, :],
                    start=(k == 0), stop=(k == D // 128 - 1),
                )
            vb = stage.tile([128, 512], F32, tag="vstage")
            nc.vector.tensor_tensor(out=vb[:], in0=pv[:, 0:512], in1=bv_sb[:],
                                    op=mybir.AluOpType.add)
            v2i = v2[:, i, :].rearrange("p (h c) -> p h c", c=65)
            nc.vector.tensor_scalar_mul(
                v2i[:, :, 0:64],
                vb[:].rearrange("p (h c) -> p h c", c=64),
                m_sb[:, i:i + 1],
            )
            nc.vector.tensor_scalar_mul(v2i[:, :, 64], ones8[:],
                                        m_sb[:, i:i + 1])

        # ---- per head-pair pipeline ----
        wqr = wq.rearrange("(k p) n -> k p n", p=128)
        wkr = wk.rearrange("(k p) n -> k p n", p=128)
        for p in range(PAIRS):
            # -- Q^T / K^T projection for this pair: [128 dcols, 2048] --
            qt = qkt_pool.tile([128, S], CDT, tag="qt")
            kt = qkt_pool.tile([128, S], CDT, tag="kt")
            for tq in range(4):  # t quarters of 512
                pp = ps_proj.tile([128, 1024], F32, tag="proj")
                for k in range(D // 128):
                    wq_c = wchunk.tile([128, 128], CDT, tag="w")
                    nc.sync.dma_start(out=wq_c[:],
                                      in_=wqr[k, :, p * 128:(p + 1) * 128])
                    nc.tensor.matmul(
                        pp[:, 0:512], wq_c[:],
                        xt[:, k, tq * 512:(tq + 1) * 512],
                        start=(k == 0), stop=(k == D // 128 - 1),
                    )
                for k in range(D // 128):
                    wk_c = wchunk.tile([128, 128], CDT, tag="w")
                    nc.sync.dma_start(out=wk_c[:],
                                      in_=wkr[k, :, p * 128:(p + 1) * 128])
                    nc.tensor.matmul(
                        pp[:, 512:1024], wk_c[:],
                        xt[:, k, tq * 512:(tq + 1) * 512],
                        start=(k == 0), stop=(k == D // 128 - 1),
                    )
                nc.vector.tensor_scalar_add(
                    qt[:, tq * 512:(tq + 1) * 512], pp[:, 0:512],
                    bq_sb[:, p:p + 1])
                nc.vector.tensor_scalar_add(
                    kt[:, tq * 512:(tq + 1) * 512], pp[:, 512:1024],
                    bk_sb[:, p:p + 1])

            # -- attention core --
            otA = ot_sb.tile([65, S], CDT, tag="ot_sb")
            otB = ot_sb.tile([65, S], CDT, tag="ot_sb")
            denA = den_pool.tile([65, S], F32, tag="den64")
            denB = den_pool.tile([65, S], F32, tag="den64")
            for qc in range(NQC):
                oA = ps_ot.tile([65, 512], F32, tag="ot")
                oB = ps_ot.tile([65, 512], F32, tag="ot")
                for i in range(NT):
                    sp = ps_s.tile([128, 1024], F32, tag="s")
                    # scores^T for both heads (row groups 0 / 64, concurrent)
                    nc.tensor.matmul(
                        sp[:, 0:512],
                        kt[0:64, i * 128:(i + 1) * 128],
                        qt[0:64, qc * 512:(qc + 1) * 512],
                        start=True, stop=True,
                    )
                    nc.tensor.matmul(
                        sp[:, 512:1024],
                        kt[64:128, i * 128:(i + 1) * 128],
                        qt[64:128, qc * 512:(qc + 1) * 512],
                        start=True, stop=True,
                    )
                    ep = e_pool.tile([128, 1024], CDT, tag="e")
                    nc.scalar.activation(ep[:], sp[:],
                                         mybir.ActivationFunctionType.Exp,
                                         scale=0.125)
                    v2i = v2[:, i, :]
                    hA = 2 * p
                    hB = 2 * p + 1
                    nc.tensor.matmul(oA[:], v2i[:, hA * 65:(hA + 1) * 65],
                                     ep[:, 0:512],
                                     start=(i == 0), stop=(i == NT - 1))
                    nc.tensor.matmul(oB[:], v2i[:, hB * 65:(hB + 1) * 65],
                                     ep[:, 512:1024],
                                     start=(i == 0), stop=(i == NT - 1))
                qs = slice(qc * 512, (qc + 1) * 512)
                nc.vector.tensor_copy(otA[0:64, qs], oA[0:64, :])
                nc.vector.tensor_copy(otB[0:64, qs], oB[0:64, :])
                nc.vector.tensor_copy(denA[64:65, qs], oA[64:65, :])
                nc.vector.tensor_copy(denB[64:65, qs], oB[64:65, :])

            # -- denominators -> [128, 16] transposed + reciprocal (fp32) --
            rTs = []
            for hs, den_t in ((0, denA), (1, denB)):
                dscr = dscratch.tile([S], F32, tag="dscr")
                nc.sync.dma_start(out=dscr[:], in_=den_t[64:65, :])
                denT = den_pool.tile([128, NT], F32, tag="denT")
                nc.sync.dma_start(
                    out=denT[:],
                    in_=dscr.rearrange("(j q) -> q j", q=128),
                )
                rT = den_pool.tile([128, NT], F32, tag="rT")
                nc.vector.reciprocal(rT[:], denT[:])
                rTs.append(rT)

            # -- transpose back + normalize + store --
            for j in range(NT):
                ott = out_pool.tile([128, 128], F32, tag="outt")
                for hs, ot_t in ((0, otA), (1, otB)):
                    ptr = ps_ot.tile([128, 64], CDT, tag="ot")
                    nc.tensor.transpose(
                        ptr[:], ot_t[0:64, j * 128:(j + 1) * 128],
                        ident[0:64, 0:64],
                    )
                    nc.vector.tensor_scalar_mul(
                        ott[:, hs * 64:(hs + 1) * 64], ptr[:],
                        rTs[hs][:, j:j + 1],
                    )
                nc.sync.dma_start(
                    out=out[j * 128:(j + 1) * 128, p * 128:(p + 1) * 128],
                    in_=ott[:],
                )


def _prep_core_inputs(c, x, mask, Wq, bq, Wk, bk, Wv, bv):
    b, hg = divmod(c, 2)
    cs = slice(hg * 512, (hg + 1) * 512)
    xTb = np.ascontiguousarray(x[b].T).astype(CNP)
    m = np.ascontiguousarray(mask[b].reshape(NT, 128).T, dtype=np.float32)
    bqc = np.ascontiguousarray(bq[cs].reshape(PAIRS, 128).T, dtype=np.float32)
    bkc = np.ascontiguousarray(bk[cs].reshape(PAIRS, 128).T, dtype=np.float32)
    bvrep = np.ascontiguousarray(
        np.broadcast_to(bv[cs][None, :], (128, 512)), dtype=np.float32)
    return {
        "xT": xTb,
        "wq": np.ascontiguousarray(Wq[:, cs]).astype(CNP),
        "wk": np.ascontiguousarray(Wk[:, cs]).astype(CNP),
        "wv": np.ascontiguousarray(Wv[:, cs]).astype(CNP),
        "mcols": m,
        "bqc": bqc,
        "bkc": bkc,
        "bvrep": bvrep,
    }


def kernel(x, mask, Wq, bq, Wk, bk, Wv, bv, _trace=False, _trace_kwargs=None):
    x = np.asarray(x, dtype=np.float32)
    mask = np.asarray(mask, dtype=np.float32)
    assert x.shape == (B, S, D) and mask.shape == (B, S)
    # every batch row must keep at least one unmasked key (softmax denominator)
    assert (mask.reshape(B, S).sum(axis=1) > 0).all()

    if "nc" not in _CACHE:
        _CACHE["nc"] = _build_program()
    nc = _CACHE["nc"]

    in_maps = [_prep_core_inputs(c, x, mask, np.asarray(Wq, np.float32),
                                 np.asarray(bq, np.float32),
                                 np.asarray(Wk, np.float32),
                                 np.asarray(bk, np.float32),
                                 np.asarray(Wv, np.float32),
                                 np.asarray(bv, np.float32))
               for c in range(NCORES)]
    kwargs = {}
    if _trace:
        kwargs["trace"] = True
        kwargs.update(_trace_kwargs or {})
    res = run_bass_kernel_spmd(nc, in_maps, core_ids=list(range(NCORES)),
                               **kwargs)
    full = np.empty((B, S, H * DH), dtype=np.float32)
    for c in range(NCORES):
        b, hg = divmod(c, 2)
        full[b, :, hg * 512:(hg + 1) * 512] = res.results[c]["out"]
    if _trace:
        kernel.last_exec_time_ns = res.exec_time_ns
        kernel.last_results = res
    return full


# revision 12
# speedup vs baseline: 1.8629x; 1.4887x over previous
"""Multi-head self-attention Trainium2 Bass kernel (8 NeuronCores).

Problem: B=4, S=2048, D=1024, H=16 heads x DH=64.
Sharding: data-parallel over batch (4) x tensor-parallel over head-groups (2)
-> 8 cores, each computing out[b, :, hg*512:(hg+1)*512].

Per-core algorithm (matmul operands bf16 -> full PE stream rate; fp32 PSUM):
  - Host supplies x[b]^T [D, S] (for Q) and a KEY-COMPACTED x[b]^T gathered at
    unmasked key positions, zero-padded to a multiple of 128 (for K and V).
    Masked keys contribute exactly zero to both the numerator and the softmax
    denominator, so dropping them is mathematically exact; compaction cuts the
    key-side work (K/V projection, scores, exp, PV) by ~the mask density.
  - Q^T, K^T computed per head-pair [128 dcols, S*] (two heads' 64 d-cols
    stacked -> row-tiled concurrent score matmuls at K=64).
  - Scores computed TRANSPOSED: S^T[t, qi] = (K^T tile).T @ Q^T -> softmax
    needs no P-transpose; exp on ACT straight from PSUM (scale=1/8 fused);
    no max-subtract needed (scores ~ N(0,1), exp cannot overflow fp32).
  - Mask folded into V: V2 = mask * [V + bv | 1]; the 65th lhsT column makes
    the PV matmul emit the masked softmax denominator for free.
  - PV: out^T[d(+den), qi] accumulated over key tiles in PSUM (fp32).
  - Epilogue: PE-transpose out^T blocks back to [qi, d], multiply by
    1/denominator (fp32, per-partition scalar), DMA to HBM.
PSUM (8 banks): scores 2x[128,1024]=4, PV-accum 2x[65,512]=2 (also reused by
the epilogue transposes), projections 1x[128,1024]=2 -> projections for the
next head-pair overlap the ACT-bound attention phase of the current pair.
"""

import os
import sys

for _p in ("/opt/trn_rl_repo", os.path.expanduser("~/.axon_site/_ro/trn_rl_repo")):
    if os.path.isdir(_p) and _p not in sys.path:
        sys.path.insert(0, _p)

import ml_dtypes
import numpy as np

import concourse.bacc as bacc
import concourse.tile as tile
from concourse import mybir
from concourse.bass_utils import run_bass_kernel_spmd
from concourse.masks import make_identity

B, S, D = 4, 2048, 1024
H, DH = 16, 64
NCORES = 8
HEADS_PER_CORE = 8
PAIRS = 4          # head pairs per core
NJ = S // 128      # 16 query tiles (output rows)
NQC = S // 512     # 4 query chunks of 512
F32 = mybir.dt.float32
CDT = mybir.dt.bfloat16          # matmul-operand compute dtype
CNP = ml_dtypes.bfloat16

_CACHE = {}


def _build_program(sc):
    """Build the SPMD Bass program; sc = padded compacted key count."""
    nc = bacc.Bacc("TRN2", target_bir_lowering=False, debug=False,
                   num_devices=NCORES)

    xT = nc.dram_tensor("xT", [D, S], CDT, kind="ExternalInput")
    xTk = nc.dram_tensor("xTk", [D, sc], CDT, kind="ExternalInput")
    wq = nc.dram_tensor("wq", [D, 512], CDT, kind="ExternalInput")
    wk = nc.dram_tensor("wk", [D, 512], CDT, kind="ExternalInput")
    wv = nc.dram_tensor("wv", [D, 512], CDT, kind="ExternalInput")
    mcols = nc.dram_tensor("mcols", [128, sc // 128], F32, kind="ExternalInput")
    bqc = nc.dram_tensor("bqc", [128, PAIRS], F32, kind="ExternalInput")
    bkc = nc.dram_tensor("bkc", [128, PAIRS], F32, kind="ExternalInput")
    bvrep = nc.dram_tensor("bvrep", [128, 512], F32, kind="ExternalInput")
    out = nc.dram_tensor("out", [S, 512], F32, kind="ExternalOutput")

    with tile.TileContext(nc) as tc:
        _emit(nc, tc, sc, xT, xTk, wq, wk, wv, mcols, bqc, bkc, bvrep, out)
    nc.compile()
    return nc


def _emit(nc, tc, sc, xT, xTk, wq, wk, wv, mcols, bqc, bkc, bvrep, out):
    from contextlib import ExitStack
    nt = sc // 128                  # key tiles (compacted)
    nkq = 4                         # query quarters for Q^T projection
    ctx = ExitStack()
    with ctx:
        consts = ctx.enter_context(tc.tile_pool(name="consts", bufs=1))
        xt_pool = ctx.enter_context(tc.tile_pool(name="xt", bufs=1))
        v2_pool = ctx.enter_context(tc.tile_pool(name="v2", bufs=1))
        qkt_pool = ctx.enter_context(tc.tile_pool(name="qkt", bufs=2))
        wchunk = ctx.enter_context(tc.tile_pool(name="wchunk", bufs=6))
        e_pool = ctx.enter_context(tc.tile_pool(name="e", bufs=3))
        ot_sb = ctx.enter_context(tc.tile_pool(name="otsb", bufs=3))
        den_pool = ctx.enter_context(tc.tile_pool(name="den", bufs=4))
        out_pool = ctx.enter_context(tc.tile_pool(name="outp", bufs=4))
        stage = ctx.enter_context(tc.tile_pool(name="stage", bufs=3))
        ps_s = ctx.enter_context(tc.tile_pool(name="ps_s", bufs=2, space="PSUM"))
        ps_ot = ctx.enter_context(tc.tile_pool(name="ps_ot", bufs=2, space="PSUM"))
        ps_proj = ctx.enter_context(tc.tile_pool(name="ps_proj", bufs=1, space="PSUM"))
        dscratch = ctx.enter_context(tc.tile_pool(name="dscr", bufs=4, space="DRAM"))

        # ---- constants / resident tensors ----
        ident = consts.tile([128, 128], CDT)
        make_identity(nc, ident[:])
        m_sb = consts.tile([128, nt], F32)
        nc.sync.dma_start(out=m_sb[:], in_=mcols[:])
        bq_sb = consts.tile([128, PAIRS], F32)
        nc.sync.dma_start(out=bq_sb[:], in_=bqc[:])
        bk_sb = consts.tile([128, PAIRS], F32)
        nc.sync.dma_start(out=bk_sb[:], in_=bkc[:])
        bv_sb = consts.tile([128, 512], F32)
        nc.sync.dma_start(out=bv_sb[:], in_=bvrep[:])
        ones8 = consts.tile([128, HEADS_PER_CORE], F32)
        nc.vector.memset(ones8[:], 1.0)
        # warm the exp table early (one-time ~2.7us load)
        warm = consts.tile([128, 16], F32)
        nc.vector.memset(warm[:], 0.0)
        nc.scalar.activation(warm[:], warm[:],
                             mybir.ActivationFunctionType.Exp, scale=1.0)

        # x^T resident (full, for Q): [128, 8, 2048]
        xt = xt_pool.tile([128, D // 128, S], CDT)
        xTr = xT.rearrange("(k p) t -> k p t", p=128)
        for k in range(D // 128):
            nc.sync.dma_start(out=xt[:, k, :], in_=xTr[k])
        # compacted x^T (for K and V): [128, 8, sc]
        xtk = xt_pool.tile([128, D // 128, sc], CDT)
        xTkr = xTk.rearrange("(k p) t -> k p t", p=128)
        for k in range(D // 128):
            nc.sync.dma_start(out=xtk[:, k, :], in_=xTkr[k])

        # Wv resident: [128, 8, 512]
        wv_sb = consts.tile([128, D // 128, 512], CDT)
        wvr = wv.rearrange("(k p) n -> k p n", p=128)
        for k in range(D // 128):
            nc.sync.dma_start(out=wv_sb[:, k, :], in_=wvr[k])

        # ---- V projection + V2 staging (all heads, compacted keys) ----
        # V2[key tile i] = [128, 8*65]: per head [V*m + bv*m | m]
        v2 = v2_pool.tile([128, nt, HEADS_PER_CORE * 65], CDT)
        for i in range(nt):
            pv = ps_proj.tile([128, 1024], F32, tag="proj")
            for k in range(D // 128):
                nc.tensor.matmul(
                    pv[:, 0:512],
                    xtk[:, k, i * 128:(i + 1) * 128],
                    wv_sb[:, k, :],
                    start=(k == 0), stop=(k == D // 128 - 1),
                )
            vb = stage.tile([128, 512], F32, tag="vstage")
            nc.vector.tensor_tensor(out=vb[:], in0=pv[:, 0:512], in1=bv_sb[:],
                                    op=mybir.AluOpType.add)
            v2i = v2[:, i, :].rearrange("p (h c) -> p h c", c=65)
            nc.vector.tensor_scalar_mul(
                v2i[:, :, 0:64],
                vb[:].rearrange("p (h c) -> p h c", c=64),
                m_sb[:, i:i + 1],
            )
            nc.vector.tensor_scalar_mul(v2i[:, :, 64], ones8[:],
                                        m_sb[:, i:i + 1])

        # ---- per head-pair pipeline ----
        wqr = wq.rearrange("(k p) n -> k p n", p=128)
        wkr = wk.rearrange("(k p) n -> k p n", p=128)
        for p in range(PAIRS):
            # -- Q^T (full queries) / K^T (compacted keys) for this pair --
            qt = qkt_pool.tile([128, S], CDT, tag="qt")
            kt = qkt_pool.tile([128, sc], CDT, tag="kt")
            wq_sb = wchunk.tile([128, D // 128, 128], CDT, tag="wqp")
            nc.sync.dma_start(out=wq_sb[:],
                              in_=wqr[:, :, p * 128:(p + 1) * 128]
                              .rearrange("k p n -> p k n"))
            wk_sb = wchunk.tile([128, D // 128, 128], CDT, tag="wkp")
            nc.sync.dma_start(out=wk_sb[:],
                              in_=wkr[:, :, p * 128:(p + 1) * 128]
                              .rearrange("k p n -> p k n"))
            for tq in range(nkq):
                pp = ps_proj.tile([128, 1024], F32, tag="proj")
                qs = slice(tq * 512, (tq + 1) * 512)
                kcnt = min(512, max(0, sc - tq * 512))
                for k in range(D // 128):
                    # interleave Q and K matmuls -> alternate PSUM banks so
                    # drains overlap the next fill
                    nc.tensor.matmul(
                        pp[:, 0:512], wq_sb[:, k, :], xt[:, k, qs],
                        start=(k == 0), stop=(k == D // 128 - 1),
                    )
                    if kcnt > 0:
                        nc.tensor.matmul(
                            pp[:, 512:512 + kcnt], wk_sb[:, k, :],
                            xtk[:, k, tq * 512:tq * 512 + kcnt],
                            start=(k == 0), stop=(k == D // 128 - 1),
                        )
                nc.vector.tensor_scalar_add(qt[:, qs], pp[:, 0:512],
                                            bq_sb[:, p:p + 1])
                if kcnt > 0:
                    nc.vector.tensor_scalar_add(
                        kt[:, tq * 512:tq * 512 + kcnt],
                        pp[:, 512:512 + kcnt], bk_sb[:, p:p + 1])

            # -- attention core --
            otA = ot_sb.tile([65, S], CDT, tag="ot_sb")
            otB = ot_sb.tile([65, S], CDT, tag="ot_sb")
            denA = den_pool.tile([65, S], F32, tag="den64")
            denB = den_pool.tile([65, S], F32, tag="den64")
            for qc in range(NQC):
                oA = ps_ot.tile([65, 512], F32, tag="ot")
                oB = ps_ot.tile([65, 512], F32, tag="ot")
                for i in range(nt):
                    sp = ps_s.tile([128, 1024], F32, tag="s")
                    # scores^T for both heads (row groups 0 / 64, concurrent)
                    nc.tensor.matmul(
                        sp[:, 0:512],
                        kt[0:64, i * 128:(i + 1) * 128],
                        qt[0:64, qc * 512:(qc + 1) * 512],
                        start=True, stop=True,
                    )
                    nc.tensor.matmul(
                        sp[:, 512:1024],
                        kt[64:128, i * 128:(i + 1) * 128],
                        qt[64:128, qc * 512:(qc + 1) * 512],
                        start=True, stop=True,
                    )
                    ep = e_pool.tile([128, 1024], CDT, tag="e")
                    nc.scalar.activation(ep[:], sp[:],
                                         mybir.ActivationFunctionType.Exp,
                                         scale=0.125)
                    v2i = v2[:, i, :]
                    hA = 2 * p
                    hB = 2 * p + 1
                    nc.tensor.matmul(oA[:], v2i[:, hA * 65:(hA + 1) * 65],
                                     ep[:, 0:512],
                                     start=(i == 0), stop=(i == nt - 1))
                    nc.tensor.matmul(oB[:], v2i[:, hB * 65:(hB + 1) * 65],
                                     ep[:, 512:1024],
                                     start=(i == 0), stop=(i == nt - 1))
                qs = slice(qc * 512, (qc + 1) * 512)
                nc.vector.tensor_copy(otA[0:64, qs], oA[0:64, :])
                nc.vector.tensor_copy(otB[0:64, qs], oB[0:64, :])
                nc.vector.tensor_copy(denA[64:65, qs], oA[64:65, :])
                nc.vector.tensor_copy(denB[64:65, qs], oB[64:65, :])

            # -- denominators -> [128, 16] transposed + reciprocal (fp32) --
            rTs = []
            for hs, den_t in ((0, denA), (1, denB)):
                dscr = dscratch.tile([S], F32, tag="dscr")
                nc.sync.dma_start(out=dscr[:], in_=den_t[64:65, :])
                denT = den_pool.tile([128, NJ], F32, tag="denT")
                nc.sync.dma_start(
                    out=denT[:],
                    in_=dscr.rearrange("(j q) -> q j", q=128),
                )
                rT = den_pool.tile([128, NJ], F32, tag="rT")
                nc.vector.reciprocal(rT[:], denT[:])
                rTs.append(rT)

            # -- transpose back + normalize + store --
            for j in range(NJ):
                ott = out_pool.tile([128, 128], F32, tag="outt")
                for hs, ot_t in ((0, otA), (1, otB)):
                    ptr = ps_ot.tile([128, 64], CDT, tag="ot")
                    nc.tensor.transpose(
                        ptr[:], ot_t[0:64, j * 128:(j + 1) * 128],
                        ident[0:64, 0:64],
                    )
                    nc.vector.tensor_scalar_mul(
                        ott[:, hs * 64:(hs + 1) * 64], ptr[:],
                        rTs[hs][:, j:j + 1],
                    )
                nc.sync.dma_start(
                    out=out[j * 128:(j + 1) * 128, p * 128:(p + 1) * 128],
                    in_=ott[:],
                )


def _prep_core_inputs(c, sc, x, mask, Wq, bq, Wk, bk, Wv, bv):
    b, hg = divmod(c, 2)
    cs = slice(hg * 512, (hg + 1) * 512)
    xTb = np.ascontiguousarray(x[b].T).astype(CNP)
    idx = np.nonzero(mask[b] > 0)[0]
    nkeys = idx.size
    xTk = np.zeros((D, sc), dtype=CNP)
    xTk[:, :nkeys] = xTb[:, idx]
    mc = np.zeros(sc, dtype=np.float32)
    mc[:nkeys] = 1.0
    mcols = np.ascontiguousarray(mc.reshape(sc // 128, 128).T)
    bqc = np.ascontiguousarray(bq[cs].reshape(PAIRS, 128).T, dtype=np.float32)
    bkc = np.ascontiguousarray(bk[cs].reshape(PAIRS, 128).T, dtype=np.float32)
    bvrep = np.ascontiguousarray(
        np.broadcast_to(bv[cs][None, :], (128, 512)), dtype=np.float32)
    return {
        "xT": xTb,
        "xTk": xTk,
        "wq": np.ascontiguousarray(Wq[:, cs]).astype(CNP),
        "wk": np.ascontiguousarray(Wk[:, cs]).astype(CNP),
        "wv": np.ascontiguousarray(Wv[:, cs]).astype(CNP),
        "mcols": mcols,
        "bqc": bqc,
        "bkc": bkc,
        "bvrep": bvrep,
    }


def kernel(x, mask, Wq, bq, Wk, bk, Wv, bv, _trace=False, _trace_kwargs=None):
    x = np.asarray(x, dtype=np.float32)
    mask = np.asarray(mask, dtype=np.float32)
    assert x.shape == (B, S, D) and mask.shape == (B, S)
    counts = (mask > 0).sum(axis=1)
    # every batch row must keep at least one unmasked key (softmax denominator)
    assert (counts > 0).all()
    sc = int(-(-int(counts.max()) // 128) * 128)

    if _CACHE.get("sc") != sc:
        _CACHE["nc"] = _build_program(sc)
        _CACHE["sc"] = sc
    nc = _CACHE["nc"]

    in_maps = [_prep_core_inputs(c, sc, x, mask, np.asarray(Wq, np.float32),
                                 np.asarray(bq, np.float32),
                                 np.asarray(Wk, np.float32),
                                 np.asarray(bk, np.float32),
                                 np.asarray(Wv, np.float32),
                                 np.asarray(bv, np.float32))
               for c in range(NCORES)]
    kwargs = {}
    if _trace:
        kwargs["trace"] = True
        kwargs.update(_trace_kwargs or {})
    res = run_bass_kernel_spmd(nc, in_maps, core_ids=list(range(NCORES)),
                               **kwargs)
    full = np.empty((B, S, H * DH), dtype=np.float32)
    for c in range(NCORES):
        b, hg = divmod(c, 2)
        full[b, :, hg * 512:(hg + 1) * 512] = res.results[c]["out"]
    if _trace:
        kernel.last_exec_time_ns = res.exec_time_ns
        kernel.last_results = res
    return full
